# revision 14
# baseline (speedup 1.0000x reference)
"""Trainium2 Bass kernel for nn_Attention_47562467836169 (Bahdanau-style attention).

Reference math (S=4096, B=128, H=64):
    q = dec @ Wq_w.T + Wq_b                      # [B,1,H]
    k = enc @ Wk_w.T + Wk_b                      # [B,S,H]
    score = tanh(q + k) @ W_w.T + W_b            # [B,S,1]
    attn = softmax(score, axis=S)
    context = attn @ enc                         # [B,1,H]

Key algorithmic facts exploited:
  * |score| <= sum|W_w| ~ 8, so softmax needs NO max-subtraction: exp() is
    safe in fp32 and the whole computation becomes a single streaming pass
    with PSUM accumulation (no second pass over the 134MB encoder tensor).
  * W_b is a constant shift -> softmax-invariant -> dropped.
  * Normalization at the end: context = (sum_s e_s * enc_s) / (sum_s e_s);
    denominator from a ones-vector matmul, applied after the loop.

Sharding: pure data parallel over B: 16 batches per core, processed as 8
stacked PAIRS (2 x 64 h-channels = 128 partitions) so every engine op uses
the full partition dim.

Per 512-row s-block on a core (streamed, triple-buffered DMA):
  per pair:
    PE transpose(enc chunks) -> PSUM -> DVE copy -> SBUF    (enc^T, h-major)
    PE k-matmul: blockdiag(WkT,WkT) @ enc^T -> z (PSUM)
    ACT tanh(z + bias2[pair]) -> SBUF      (bias2 = (dec@WqT + Wq_b + Wk_b)^T,
                                            computed on device in a preamble)
    PE v-matmuls: tanh^T(stationary) x [v|0,0|v] -> scores s-on-partition
  ACT exp over all pairs' scores at once -> e [128s, 64]
  PE ctx-matmuls: e-slices^T @ enc -> [8, 512] per bank, PSUM-accumulated
  PE den-matmul: ones^T @ e -> [1, 64], DVE-accumulated in SBUF
Tail: evacuate, reduce+transpose denominator, reciprocal, scale, DMA out.
"""

import numpy as np

S, B, H = 4096, 128, 64
NCORES = 8
BC = B // NCORES          # batches per core = 16
PAIRS = BC // 2           # 8
PW = 2 * H                # per-pair free width in the enc layout = 128
FW = PAIRS * PW           # enc free width per s-row = 1024
SBLK = 512                # s rows per DMA block
NQ = SBLK // 128          # 128-row chunks per block = 4
NBLK = S // SBLK          # 8

_CACHE = {}


def _build_nc(nblk):
    import concourse.bacc as bacc
    import concourse.tile as tile
    from concourse import mybir

    f32 = mybir.dt.float32
    Act = mybir.ActivationFunctionType
    s_total = nblk * SBLK

    nc = bacc.Bacc(None, target_bir_lowering=False)
    enc_d = nc.dram_tensor("enc", [s_total, FW], f32, kind="ExternalInput")
    decT_d = nc.dram_tensor("dect", [H + 2, BC], f32, kind="ExternalInput")
    wqT_d = nc.dram_tensor("wqt", [H + 2, H], f32, kind="ExternalInput")
    wk2_d = nc.dram_tensor("wk2", [128, 128], f32, kind="ExternalInput")
    v2_d = nc.dram_tensor("v2", [128, 2], f32, kind="ExternalInput")
    id_d = nc.dram_tensor("ident", [128, 128], f32, kind="ExternalInput")
    out_d = nc.dram_tensor("outp", [2, 8, 512], f32, kind="ExternalOutput")

    with tile.TileContext(nc) as tc:
        with tc.tile_pool(name="singles", bufs=1) as singles:
            wk2_sb = singles.tile([128, 128], f32)
            v2_sb = singles.tile([128, 2], f32)
            id_sb = singles.tile([128, 128], f32)
            decT_sb = singles.tile([H + 2, BC], f32)
            wqT_sb = singles.tile([H + 2, H], f32)
            bias2_sb = singles.tile([128, PAIRS], f32)
            ones_sb = singles.tile([128, 1], f32)
            den_acc = singles.tile([1, 8 * PAIRS], f32)
            den16_sb = singles.tile([1, BC], f32)
            rden_sb = singles.tile([PAIRS, 2], f32)
            ctx_sb = singles.tile([PAIRS, 2, 512], f32)
            nc.vector.memset(ones_sb[:], 1.0)
            nc.vector.memset(den_acc[:], 0.0)
            nc.sync.dma_start(wk2_sb[:], wk2_d[:])
            nc.sync.dma_start(v2_sb[:], v2_d[:])
            nc.sync.dma_start(id_sb[:], id_d[:])
            nc.sync.dma_start(decT_sb[:], decT_d[:])
            nc.sync.dma_start(wqT_sb[:], wqT_d[:])

            # ---- preamble: bias2[64*j + h', p] = (dec[2p+j] @ WqT + Wq_b + Wk_b)[h']
            # Computed directly transposed: bias2_half = wqT_aug^T-contract @ decT
            # (out[h', p] = sum_k wqT[k, h'] * decT[k, p]).
            with tc.tile_pool(name="pre_ps", bufs=1, space="PSUM") as pre_ps:
                bias2_ps = pre_ps.tile([128, PAIRS], f32)
                nc.tensor.matmul(bias2_ps[0:H, :], wqT_sb[:], decT_sb[:, 0:PAIRS],
                                 start=True, stop=True)
                nc.tensor.matmul(bias2_ps[H:2 * H, :], wqT_sb[:],
                                 decT_sb[:, PAIRS:BC], start=True, stop=True,
                                 tile_position=(0, 64), skip_group_check=True)
                nc.scalar.copy(bias2_sb[:], bias2_ps[:])

            # ---- main streaming loop
            enc_r = enc_d[:].rearrange("(nb q p) f -> nb p q f", q=NQ, p=128)
            with tc.tile_pool(name="psC", bufs=1, space="PSUM") as psC:
                ctx_ps0 = psC.tile([PAIRS, 512], f32, tag="ctx0")
                ctx_ps1 = psC.tile([PAIRS, 512], f32, tag="ctx1")
                ctx_ps = [ctx_ps0, ctx_ps1]
                with (
                    tc.tile_pool(name="eblk", bufs=3) as eblk,
                    tc.tile_pool(name="work", bufs=3) as work,
                    tc.tile_pool(name="epool", bufs=3) as epool,
                    tc.tile_pool(name="psA", bufs=2, space="PSUM") as psA,
                    tc.tile_pool(name="psB", bufs=2, space="PSUM") as psB,
                    tc.tile_pool(name="psS", bufs=2, space="PSUM") as psS,
                ):
                  for nb in range(nblk):
                    etile = eblk.tile([128, NQ, FW], f32, tag="etile")
                    nc.sync.dma_start(etile[:], enc_r[nb])
                    sc_ps = psS.tile([128, 8 * PAIRS], f32, tag="sc")
                    e_sb = epool.tile([128, 8 * PAIRS], f32, tag="e")
                    for p in range(PAIRS):
                        et2_ps = psA.tile([128, SBLK], f32, tag="et2")
                        for q in range(NQ):
                            nc.tensor.transpose(
                                et2_ps[:, 128 * q:128 * (q + 1)],
                                etile[:, q, PW * p:PW * (p + 1)],
                                id_sb[:],
                            )
                        et2_sb = work.tile([128, SBLK], f32, tag="et2sb")
                        nc.vector.tensor_copy(et2_sb[:], et2_ps[:])
                        z_ps = psB.tile([128, SBLK], f32, tag="z")
                        nc.tensor.matmul(z_ps[:], wk2_sb[:], et2_sb[:],
                                         start=True, stop=True)
                        th_sb = work.tile([128, SBLK], f32, tag="th")
                        nc.scalar.activation(th_sb[:], z_ps[:], Act.Tanh,
                                             bias=bias2_sb[:, p:p + 1], scale=1.0)
                        for q in range(NQ):
                            c0 = 16 * q + 2 * p
                            nc.tensor.matmul(
                                sc_ps[:, c0:c0 + 2],
                                th_sb[:, 128 * q:128 * (q + 1)],
                                v2_sb[:],
                                start=True, stop=True,
                            )
                    nc.scalar.activation(e_sb[:], sc_ps[:], Act.Exp)
                    # scores col layout: 16q + 2p + j
                    for g in range(2):
                        for q in range(NQ):
                            nc.tensor.matmul(
                                ctx_ps[g][:, :],
                                e_sb[:, 16 * q + 8 * g:16 * q + 8 * g + 8],
                                etile[:, q, 512 * g:512 * (g + 1)],
                                start=(nb == 0 and q == 0),
                                stop=(nb == nblk - 1 and q == NQ - 1),
                                skip_group_check=True,
                            )
                    # denominator partials: ones^T @ e -> [1, 64]
                    den_ps = psS.tile([1, 8 * PAIRS], f32, tag="sc")
                    nc.tensor.matmul(den_ps[:], ones_sb[:], e_sb[:],
                                     start=True, stop=True)
                    nc.vector.tensor_tensor(den_acc[:], den_acc[:], den_ps[:],
                                            op=mybir.AluOpType.add)

                # ---- tail: denominator -> per-batch reciprocal on partitions
                den_r = den_acc[:].rearrange("o (q p j) -> o p j q", q=NQ, j=2)
                nc.vector.tensor_reduce(
                    den16_sb[:].rearrange("o (p j) -> o p j", j=2), den_r,
                    axis=mybir.AxisListType.X, op=mybir.AluOpType.add)
                with tc.tile_pool(name="post_ps", bufs=1, space="PSUM") as post_ps:
                    rden_ps = post_ps.tile([PAIRS, 2], f32)
                    nc.tensor.transpose(rden_ps[:, 0:1], den16_sb[0:1, 0:PAIRS],
                                        id_sb[0:1, 0:1])
                    nc.tensor.transpose(rden_ps[:, 1:2], den16_sb[0:1, PAIRS:BC],
                                        id_sb[0:1, 0:1])
                    nc.vector.reciprocal(rden_sb[:], rden_ps[:])
                    for g in range(2):
                        nc.scalar.copy(ctx_sb[:, g, :], ctx_ps[g][:, :])
                        nc.vector.tensor_scalar_mul(ctx_sb[:, g, :], ctx_sb[:, g, :],
                                                    rden_sb[:, g:g + 1])
                        nc.sync.dma_start(out_d[g], ctx_sb[:, g, :])
    nc.compile()
    return nc


def get_nc(nblk=NBLK):
    if nblk not in _CACHE:
        _CACHE[nblk] = _build_nc(nblk)
    return _CACHE[nblk]


def host_prep(enc, dec, wq_w, wq_b, wk_w, wk_b, w_w, nblk=NBLK):
    """Build the 8 per-core input maps. enc [S',B,H] f32, dec [B,H]."""
    s_total = nblk * SBLK
    wk2 = np.zeros((128, 128), np.float32)
    wk2[0:H, 0:H] = wk_w.T
    wk2[H:2 * H, H:2 * H] = wk_w.T
    v2 = np.zeros((128, 2), np.float32)
    v2[0:H, 0] = w_w[0]
    v2[H:2 * H, 1] = w_w[0]
    ident = np.eye(128, dtype=np.float32)
    wqT = np.zeros((H + 2, H), np.float32)
    wqT[0:H] = wq_w.T
    wqT[H] = wq_b
    wqT[H + 1] = wk_b
    in_maps = []
    for c in range(NCORES):
        e = enc[:, BC * c:BC * (c + 1), :]            # [S', 16, 64]
        buf = np.empty((s_total, PAIRS, PW), np.float32)
        buf[:, :, 0:H] = e[:, 0::2, :]
        buf[:, :, H:2 * H] = e[:, 1::2, :]
        d = dec[BC * c:BC * (c + 1)]                  # [16, 64]
        decT = np.ones((H + 2, BC), np.float32)
        decT[0:H, 0:PAIRS] = d[0::2].T
        decT[0:H, PAIRS:BC] = d[1::2].T
        in_maps.append({
            "enc": np.ascontiguousarray(buf.reshape(s_total, FW)),
            "dect": decT, "wqt": wqT, "wk2": wk2, "v2": v2, "ident": ident,
        })
    return in_maps


def assemble_output(results):
    """results: list of 8 dicts with 'outp' [2,8,512] -> full [1,B,H]."""
    out = np.zeros((1, B, H), np.float32)
    for c in range(NCORES):
        o = results[c]["outp"]
        for g in range(2):
            for a in range(4):
                for j in range(2):
                    b = BC * c + 2 * (4 * g + a) + j
                    out[0, b, :] = o[g, 2 * a + j, 128 * a + H * j:128 * a + H * (j + 1)]
    return out


def kernel(encoder_outputs, decoder_hidden, Wq_w, Wq_b, Wk_w, Wk_b, W_w, W_b,
           **kwargs):
    from concourse.bass_utils import run_bass_kernel_spmd

    enc = np.asarray(encoder_outputs, np.float32)
    dec = np.asarray(decoder_hidden, np.float32)[0]
    in_maps = host_prep(enc, dec,
                        np.asarray(Wq_w, np.float32), np.asarray(Wq_b, np.float32),
                        np.asarray(Wk_w, np.float32), np.asarray(Wk_b, np.float32),
                        np.asarray(W_w, np.float32))
    nc = get_nc()
    res = run_bass_kernel_spmd(nc, in_maps, core_ids=list(range(NCORES)))
    return assemble_output(res.results)
